# revision 8
# baseline (speedup 1.0000x reference)
"""Trainium2 Bass kernel for nn_PointCloudLoss (chamfer + sinkhorn EMD).

Strategy
--------
Data-parallel over batch: 16 batches -> 8 cores x 2 batches/core.

Per batch the sinkhorn iteration is computed without ever materializing the
[N,N] cost matrix in SBUF: each half-iteration's exp argument

    A[n,m] = gs[m] + lb - C[n,m]/eps        (C = 0.5*|x_n - y_m|^2)

is produced directly in PSUM by a single K=11 fp16 matmul (the x.y/eps term
uses an fp16 hi/lo split for fp32-grade accuracy; the m-indexed terms ride in
two dynamic rhs rows; the n-indexed terms go into the ScalarE activation bias
together with a per-row stabilization shift).  One ScalarE Exp activation per
PSUM tile then computes exp and its per-row sum (accum_out) in one pass.
The per-row shift is seeded exactly at iteration 1 by a VectorE reduce_max of
the PSUM tiles and afterwards updated by shift += ln(S) (the running shift is
then exactly -f/eps, which both stabilizes every later exp pass and *is* the
final potential).  Chamfer row/col minima come for free from the iteration-1
row maxima.  Host only splits/packs inputs and averages the tiny outputs.

Point-to-slot mapping n = p*16 + t (partition p, tile t) makes every
per-iteration [128,16] -> [1,2048] vector relayout a plain contiguous DMA.
"""

import numpy as np

import concourse.bacc as bacc
import concourse.tile as tile
from concourse import mybir
from concourse.bass_utils import run_bass_kernel_spmd

N = 2048
B = 16
NCORES = 8
BPC = B // NCORES          # batches per core
EPS = np.float32(0.05 ** 2)
ITERS = 50
NT = N // 128              # 16 row tiles
NC_CHUNK = 512             # matmul moving free dim (one PSUM bank)
K = 11                     # contraction rows of the fused matmul
LOG_N = np.float32(np.log(np.float32(N)))
LA = np.float32(-LOG_N)    # log(1/N)
LB = np.float32(-LOG_N)

F32 = mybir.dt.float32
F16 = mybir.dt.float16
AF = mybir.ActivationFunctionType
AX = mybir.AxisListType


def _emit_pass(nc, tc, pools, st, first):
    """One sinkhorn half-iteration (one direction) for one batch.

    st: dict with this direction's tiles:
      lhs   [11, 2048] f16   stationary rows (this direction's lhsT data)
      rhs   [11, 2048] f16   moving rows (rows 9,10 dynamic)
      bstat [128,16] f32     -xx/(2eps)   (own-side static bias part)
      rstat [128,16] f32     const + other-side static part for the r rows
      shift [128,16] f32     running shift state (== -potential/eps)
      bias  [128,16] f32     activation bias for this direction
      S     [128,16] f32     row sums
      mneg  [128,16] f32     (first pass only) negated row maxima
      rhs_other              rhs tile of the *other* direction (rows 9,10 target)
    """
    psum_pool, small_pool = pools
    lhs, rhs = st["lhs"], st["rhs"]

    for t in range(NT):
        ps = psum_pool.tile([128, N], F32, tag="ps")
        for c in range(N // NC_CHUNK):
            nc.tensor.matmul(
                ps[:, c * NC_CHUNK:(c + 1) * NC_CHUNK],
                lhs[0:K, t * 128:(t + 1) * 128],
                rhs[0:K, c * NC_CHUNK:(c + 1) * NC_CHUNK],
                start=True, stop=True,
            )
        if first:
            # exact per-row shift seed (also yields chamfer minima)
            nc.vector.tensor_reduce(
                st["mneg"][:, t:t + 1], ps[:, 0:N], axis=AX.X,
                op=mybir.AluOpType.max, negate=True,
            )
            bias_ap = st["mneg"][:, t:t + 1]
        else:
            bias_ap = st["bias"][:, t:t + 1]
        nc.scalar.activation(
            ps[:, 0:N], ps[:, 0:N], AF.Exp,
            bias=bias_ap, scale=1.0,
            accum_out=st["S"][:, t:t + 1],
        )

    # ---- small per-pass epilogue (partition-major [128,16] ops) ----
    lnS = small_pool.tile([128, NT], F32, tag="lnS")
    nc.scalar.activation(lnS[:, :], st["S"][:, :], AF.Ln)
    if first:
        # shift = -mneg + bstat ; then shift += lnS
        nc.vector.tensor_sub(st["shift"][:, :], st["bstat"][:, :], st["mneg"][:, :])
        nc.vector.tensor_add(st["shift"][:, :], st["shift"][:, :], lnS[:, :])
    else:
        nc.vector.tensor_add(st["shift"][:, :], st["shift"][:, :], lnS[:, :])
    # next-pass activation bias for this direction
    nc.vector.tensor_sub(st["bias"][:, :], st["bstat"][:, :], st["shift"][:, :])
    # dynamic r rows for the *other* direction: r = rstat - shift, split hi/lo
    rp = small_pool.tile([128, NT], F32, tag="rp")
    nc.vector.tensor_sub(rp[:, :], st["rstat"][:, :], st["shift"][:, :])
    rph = small_pool.tile([128, NT], F16, tag="rph")
    rpl = small_pool.tile([128, NT], F16, tag="rpl")
    nc.vector.tensor_copy(rph[:, :], rp[:, :])
    nc.vector.tensor_sub(rpl[:, :], rp[:, :], rph[:, :])
    ro = st["rhs_other"]
    nc.sync.dma_start(ro[9:10, 0:N], rph[:, :])
    nc.sync.dma_start(ro[10:11, 0:N], rpl[:, :])


def _build_nc(iters=ITERS, use_for_i=True):
    nc = bacc.Bacc("TRN2", target_bir_lowering=False, debug=False)

    dram = {}
    for j in range(BPC):
        for nm, shape, dt in [
            (f"lhsf{j}", [K, N], F16), (f"rhsf{j}", [K, N], F16),
            (f"lhsg{j}", [K, N], F16), (f"rhsg{j}", [K, N], F16),
            (f"rhsg0{j}", [K, N], F16),
            (f"bstat{j}", [128, 2 * NT], F32),
            (f"rstat{j}", [128, 2 * NT], F32),
        ]:
            dram[nm] = nc.dram_tensor(nm, shape, dt, kind="ExternalInput")
        dram[f"out{j}"] = nc.dram_tensor(f"out{j}", [128, 4 * NT], F32,
                                         kind="ExternalOutput")

    with tile.TileContext(nc) as tc:
        import contextlib
        with contextlib.ExitStack() as ctx:
            psum_pool = ctx.enter_context(
                tc.tile_pool(name="psum", bufs=2, space="PSUM"))
            small_pool = ctx.enter_context(tc.tile_pool(name="small", bufs=2))
            const_pool = ctx.enter_context(tc.tile_pool(name="const", bufs=1))
            pools = (psum_pool, small_pool)

            sts = []
            for j in range(BPC):
                lhsf = const_pool.tile([K, N], F16, tag=f"lhsf{j}")
                rhsf = const_pool.tile([K, N], F16, tag=f"rhsf{j}")
                lhsg = const_pool.tile([K, N], F16, tag=f"lhsg{j}")
                rhsg = const_pool.tile([K, N], F16, tag=f"rhsg{j}")
                rhsg0 = const_pool.tile([K, N], F16, tag=f"rhsg0{j}")
                bstat = const_pool.tile([128, 2 * NT], F32, tag=f"bstat{j}")
                rstat = const_pool.tile([128, 2 * NT], F32, tag=f"rstat{j}")
                for tl, nm in [(lhsf, f"lhsf{j}"), (rhsf, f"rhsf{j}"),
                               (lhsg, f"lhsg{j}"), (rhsg, f"rhsg{j}"),
                               (rhsg0, f"rhsg0{j}"), (bstat, f"bstat{j}"),
                               (rstat, f"rstat{j}")]:
                    nc.sync.dma_start(tl[:, :], dram[nm].ap())

                def mk(nmn):
                    return const_pool.tile([128, NT], F32, tag=f"{nmn}{j}",
                                           name=f"{nmn}{j}")
                stf = dict(lhs=lhsf, rhs=rhsf, bstat=bstat[:, 0:NT],
                           rstat=rstat[:, 0:NT], shift=mk("shf"), bias=mk("bif"),
                           S=mk("Sf"), mneg=mk("mnf"), rhs_other=rhsg)
                stg = dict(lhs=lhsg, rhs=rhsg, bstat=bstat[:, NT:2 * NT],
                           rstat=rstat[:, NT:2 * NT], shift=mk("shg"),
                           bias=mk("big"), S=mk("Sg"), mneg=mk("mng"),
                           rhs_other=rhsf)
                # pure transposed pass state for column minima (no exp needed)
                stp = dict(lhs=lhsg, rhs=rhsg0, mneg=mk("mnp"))
                sts.append((stf, stg, stp))

            # ---- iteration 1 (peeled: exact shift seeding + chamfer) ----
            for j in range(BPC):
                stf, _, _ = sts[j]
                _emit_pass(nc, tc, pools, stf, first=True)
            for j in range(BPC):
                stf, _, stp = sts[j]
                # pure pass: column minima of C (rhs has no potential rows)
                for t in range(NT):
                    ps = psum_pool.tile([128, N], F32, tag="ps")
                    for c in range(N // NC_CHUNK):
                        nc.tensor.matmul(
                            ps[:, c * NC_CHUNK:(c + 1) * NC_CHUNK],
                            stp["lhs"][0:K, t * 128:(t + 1) * 128],
                            stp["rhs"][0:K, c * NC_CHUNK:(c + 1) * NC_CHUNK],
                            start=True, stop=True,
                        )
                    nc.vector.tensor_reduce(
                        stp["mneg"][:, t:t + 1], ps[:, 0:N], axis=AX.X,
                        op=mybir.AluOpType.max, negate=True,
                    )
                nc.sync.dma_start(dram[f"out{j}"].ap()[:, 0:NT], stf["mneg"][:, :])
                nc.sync.dma_start(dram[f"out{j}"].ap()[:, NT:2 * NT],
                                  stp["mneg"][:, :])
            for j in range(BPC):
                _, stg, _ = sts[j]
                _emit_pass(nc, tc, pools, stg, first=True)

            # ---- iterations 2..iters ----
            def body(_i=None):
                for j in range(BPC):
                    _emit_pass(nc, tc, pools, sts[j][0], first=False)
                for j in range(BPC):
                    _emit_pass(nc, tc, pools, sts[j][1], first=False)

            nrem = iters - 1
            if use_for_i and nrem > 0:
                with tc.For_i(0, nrem, 1,
                              hint_engines=(mybir.EngineType.PE,)):
                    body()
            else:
                for _ in range(nrem):
                    body()

            for j in range(BPC):
                stf, stg, _ = sts[j]
                nc.sync.dma_start(dram[f"out{j}"].ap()[:, 2 * NT:3 * NT],
                                  stf["shift"][:, :])
                nc.sync.dma_start(dram[f"out{j}"].ap()[:, 3 * NT:4 * NT],
                                  stg["shift"][:, :])

    nc.compile()
    return nc


def _split16(a):
    h = a.astype(np.float16)
    l = (a.astype(np.float32) - h.astype(np.float32)).astype(np.float16)
    return h, l


def _tilemap_cols(rows):
    # rows [K, N] indexed by point n -> matmul lhsT column order 128*t + p
    # with n = p*16 + t
    kk = rows.shape[0]
    return rows.reshape(kk, 128, NT).transpose(0, 2, 1).reshape(kk, N).copy()


def _host_prep(x, y):
    """Build one batch's DRAM input arrays. x,y: [N,3] float32."""
    eps = EPS
    xx = (x * x).sum(-1).astype(np.float32)
    yy = (y * y).sum(-1).astype(np.float32)
    U = (x.T / eps).astype(np.float32)     # [3,N]
    V = y.T.astype(np.float32)
    W = (y.T / eps).astype(np.float32)
    X2 = x.T.astype(np.float32)
    uh, ul = _split16(U); vh, vl = _split16(V)
    wh, wl = _split16(W); xh, xl = _split16(X2)
    ones = np.ones((1, N), np.float16)
    zeros = np.zeros((1, N), np.float16)

    def lhs_rows(h, l):
        return np.concatenate([h[0:1], h[0:1], l[0:1],
                               h[1:2], h[1:2], l[1:2],
                               h[2:3], h[2:3], l[2:3], ones, ones], 0)

    def rhs_rows(h, l, r9, r10):
        return np.concatenate([h[0:1], l[0:1], h[0:1],
                               h[1:2], l[1:2], h[1:2],
                               h[2:3], l[2:3], h[2:3], r9, r10], 0)

    bxx = (-xx / (2 * eps)).astype(np.float32)
    byy = (-yy / (2 * eps)).astype(np.float32)
    r0 = (LB + byy).astype(np.float32)          # initial r rows (gs = 0)
    r0h, r0l = _split16(r0)
    rp0 = (LA + bxx).astype(np.float32)         # pure-pass rows (fs = 0)
    rp0h, rp0l = _split16(rp0)

    d = {
        "lhsf": _tilemap_cols(lhs_rows(uh, ul)),
        "rhsf": rhs_rows(vh, vl, r0h[None, :], r0l[None, :]),
        "lhsg": _tilemap_cols(lhs_rows(wh, wl)),
        "rhsg": rhs_rows(xh, xl, zeros, zeros),
        "rhsg0": rhs_rows(xh, xl, rp0h[None, :], rp0l[None, :]),
        "bstat": np.concatenate([bxx.reshape(128, NT), byy.reshape(128, NT)],
                                1).astype(np.float32),
        "rstat": np.concatenate([(LA + bxx).reshape(128, NT),
                                 (LB + byy).reshape(128, NT)],
                                1).astype(np.float32),
    }
    return d, xx, yy


_CACHE = {}


def _get_nc():
    if "nc" not in _CACHE:
        _CACHE["nc"] = _build_nc()
    return _CACHE["nc"]


def _run(y_true, y_pred, trace=False):
    y_true = np.asarray(y_true, np.float32).reshape(B, N, 3)
    y_pred = np.asarray(y_pred, np.float32).reshape(B, N, 3)
    nc = _get_nc()

    in_maps = []
    xxs, yys = [], []
    for core in range(NCORES):
        m = {}
        for j in range(BPC):
            b = core * BPC + j
            d, xx, yy = _host_prep(y_true[b], y_pred[b])
            xxs.append(xx); yys.append(yy)
            for k, v in d.items():
                m[f"{k}{j}"] = v
        in_maps.append(m)

    res = run_bass_kernel_spmd(nc, in_maps, core_ids=list(range(NCORES)),
                               trace=trace)
    results = res.results

    eps = float(EPS)
    cd_b = np.zeros(B, np.float64)
    emd_b = np.zeros(B, np.float64)
    for core in range(NCORES):
        for j in range(BPC):
            b = core * BPC + j
            o = np.asarray(results[core][f"out{j}"], np.float32)
            mneg_f = o[:, 0:NT].reshape(N)
            mneg_g0 = o[:, NT:2 * NT].reshape(N)
            shf = o[:, 2 * NT:3 * NT].reshape(N)
            shg = o[:, 3 * NT:4 * NT].reshape(N)
            minrow_c = eps * (float(LB) + mneg_f) + xxs[b] / 2.0
            mincol_c = eps * (float(LA) + mneg_g0) + yys[b] / 2.0
            f = -eps * shf
            g = -eps * shg
            cd_b[b] = 2.0 * minrow_c.mean() + 2.0 * mincol_c.mean()
            emd_b[b] = f.mean() + g.mean()
    cd = cd_b.mean()
    out = 0.5 * cd + 0.5 * emd_b
    return out.astype(np.float32), res


def kernel(y_true, y_pred):
    out, _ = _run(y_true, y_pred)
    return out
